# revision 8
# baseline (speedup 1.0000x reference)
"""3-layer GCN (SpMM -> GEMM -> bias -> relu, x3) on 8 Trainium2 NeuronCores.

Strategy (pull-mode graph parallelism):
  - Destination nodes are sharded across the 8 cores (12500 rows each); edges
    are partitioned by destination core.  W/b weights are replicated.
  - Each layer's node-feature table is replicated in every core's DRAM,
    split into NCHUNK row-chunks so dma_gather's int16 indices can address
    any row.  Chunk q holds, for every rank r, rank r's q-th quarter of
    rows, which is exactly what a per-quarter AllGather produces.
  - SpMM is computed per 128/256-wide destination tile: gather the source
    rows for the tile's edges (dma_gather), build a weighted one-hot
    [edges x T] matrix on the Vector engine (iota == dst_local) * w, and
    accumulate  Y^T[feat, dst] += G[edges, feat].T @ onehot[edges, dst]
    on the TensorEngine in PSUM.
  - The dense layer is then  H[dst, dout] = (Y^T).T @ W  with Y^T as the
    stationary operand, bias added on the Vector engine, ReLU on Scalar.
  - Between layers the per-core row slices are AllGathered (one collective
    per chunk, so communication overlaps the tail of the layer).
"""

import math

import numpy as np

import concourse.bacc as bacc
import concourse.mybir as mybir
import concourse.tile as tile
from concourse.bass_utils import run_bass_kernel_spmd

F32 = mybir.dt.float32
F32R = mybir.dt.float32r
BF16 = mybir.dt.bfloat16
I16 = mybir.dt.int16


class Cfg:
    def __init__(
        self,
        n_nodes,
        d_in,
        d_hid,
        d_out,
        n_cores=8,
        tile_w=256,
        grp=2,
        n_chunks=4,
        dt="f32",
        use_f32r=True,
        gbufs=2,
    ):
        self.N = n_nodes
        self.DIN = d_in
        self.DHID = d_hid
        self.DOUT = d_out
        self.C = n_cores
        assert n_nodes % n_cores == 0
        self.ROWS = n_nodes // n_cores
        assert self.ROWS % n_chunks == 0
        self.NCHUNK = n_chunks
        self.QUARTER = self.ROWS // n_chunks
        self.CHUNK_ROWS = n_nodes // n_chunks
        assert self.CHUNK_ROWS <= 32767, "dma_gather idx is int16"
        self.T = tile_w
        self.GRP = grp
        self.NT = math.ceil(self.ROWS / tile_w)
        self.NG = math.ceil(self.NT / grp)
        self.dt = dt
        self.mdt = F32 if dt == "f32" else BF16
        self.np_mdt = np.float32 if dt == "f32" else None  # set below
        self.use_f32r = use_f32r and dt == "f32"
        self.gbufs = gbufs
        if dt != "f32":
            import ml_dtypes

            self.np_mdt = ml_dtypes.bfloat16


def _prep(cfg, edge_src, edge_dst, edge_weight):
    """Host-side edge preprocessing.  Returns per-core input arrays plus the
    static (core-independent) block structure."""
    src = np.asarray(edge_src, dtype=np.int64)
    dst = np.asarray(edge_dst, dtype=np.int64)
    w = np.asarray(edge_weight, dtype=np.float32)

    C, ROWS, T, GRP, Q = cfg.C, cfg.ROWS, cfg.T, cfg.GRP, cfg.QUARTER
    NT, NG, NCHUNK = cfg.NT, cfg.NG, cfg.NCHUNK

    core = dst // ROWS
    dstl_all = dst - core * ROWS
    tl_all = dstl_all // T
    grp_all = tl_all // GRP
    q_all = (src % ROWS) // Q
    cidx_all = (src // ROWS) * Q + (src % Q)

    # counts[c, g, q, t_local]
    counts = np.zeros((C, NG, NCHUNK, GRP), dtype=np.int64)
    tloc_all = tl_all - grp_all * GRP
    np.add.at(
        counts,
        (core, grp_all, q_all, tloc_all),
        1,
    )
    # static region sizes: equal across cores, multiple of 128 per region
    region = counts.max(axis=0)  # [NG, NCHUNK, GRP]
    region = ((region + 127) // 128) * 128
    # tiles past NT contribute nothing (last group may be partial)
    for g in range(NG):
        for tl in range(GRP):
            if g * GRP + tl >= NT:
                region[g, :, tl] = 0

    seg_len = region.sum(axis=2)  # [NG, NCHUNK] slots per gather call
    group_slots = seg_len.sum(axis=1)  # [NG]
    slot_base = np.concatenate([[0], np.cumsum(group_slots)])
    total_slots = int(slot_base[-1])

    # slot offset of region (g, q, t_local)
    reg_off = np.zeros((NG, NCHUNK, GRP), dtype=np.int64)
    for g in range(NG):
        off = slot_base[g]
        for q in range(NCHUNK):
            for tl in range(GRP):
                reg_off[g, q, tl] = off
                off += region[g, q, tl]

    # per-core slot arrays
    per_core = []
    for c in range(C):
        m = core == c
        e_cidx = cidx_all[m]
        e_q = q_all[m]
        e_g = grp_all[m]
        e_tl = tloc_all[m]
        e_dstl = dstl_all[m] - (e_g * GRP + e_tl) * T  # local within tile
        e_w = w[m]
        order = np.lexsort((e_tl, e_q, e_g))
        e_cidx, e_q, e_g, e_tl, e_dstl, e_w = (
            a[order] for a in (e_cidx, e_q, e_g, e_tl, e_dstl, e_w)
        )
        # position within its (g,q,t) region
        cnt = counts[c, e_g, e_q, e_tl]
        # running index within each sorted group-run
        # since sorted by (g,q,t), positions are 0..cnt-1 per run
        run_starts = np.flatnonzero(
            np.concatenate(
                [[True], (np.diff(e_g) != 0) | (np.diff(e_q) != 0) | (np.diff(e_tl) != 0)]
            )
        )
        pos = np.arange(len(e_g)) - np.repeat(
            run_starts, np.diff(np.concatenate([run_starts, [len(e_g)]]))
        )
        assert (pos < cnt).all()
        slots = reg_off[e_g, e_q, e_tl] + pos

        gidx = np.zeros(total_slots, dtype=np.int16)
        dstl_arr = np.full(total_slots, -1.0, dtype=np.float32)
        w_arr = np.zeros(total_slots, dtype=np.float32)
        gidx[slots] = e_cidx.astype(np.int16)
        dstl_arr[slots] = e_dstl.astype(np.float32)
        w_arr[slots] = e_w
        per_core.append((gidx, dstl_arr, w_arr))

    # pack into DMA-ready layouts
    total_blocks = total_slots // 128
    inputs = []
    for c in range(C):
        gidx, dstl_arr, w_arr = per_core[c]
        # idx: per (g,q) segment, wrapped [16, L/16], replicated to 128 parts
        idx_plane = np.zeros((128, total_slots // 16), dtype=np.int16)
        for g in range(NG):
            off = int(slot_base[g])
            for q in range(NCHUNK):
                L = int(seg_len[g, q])
                if L == 0:
                    continue
                seg = gidx[off : off + L]
                wrapped = seg.reshape(L // 16, 16).T  # [16, L/16]
                idx_plane[:, off // 16 : (off + L) // 16] = np.tile(wrapped, (8, 1))
                off += L
        dstl_plane = dstl_arr.reshape(total_blocks, 128).T.astype(cfg.np_mdt)
        w_plane = w_arr.reshape(total_blocks, 128).T.astype(cfg.np_mdt)
        inputs.append(
            {"IDX": idx_plane, "DSTL": np.ascontiguousarray(dstl_plane),
             "WGT": np.ascontiguousarray(w_plane)}
        )

    meta = {
        "region": region,
        "seg_len": seg_len,
        "group_slots": group_slots,
        "slot_base": slot_base,
        "reg_off": reg_off,
        "total_slots": total_slots,
        "total_blocks": total_blocks,
    }
    return inputs, meta


def _chunk_tables(cfg, h):
    """Rearrange full [N, D] host array into NCHUNK chunk tables
    [CHUNK_ROWS, D]: chunk q row (r*QUARTER + j) = node r*ROWS + q*QUARTER + j."""
    C, ROWS, Q = cfg.C, cfg.ROWS, cfg.QUARTER
    tabs = []
    for q in range(cfg.NCHUNK):
        rows = np.concatenate(
            [h[r * ROWS + q * Q : r * ROWS + (q + 1) * Q] for r in range(C)], axis=0
        )
        tabs.append(np.ascontiguousarray(rows))
    return tabs


def _build(nc, cfg, meta, linearize=False, layers=(0, 1, 2), skip_ag=False):
    """Emit the 3-layer GCN program for one core (SPMD: all cores identical)."""
    mdt = cfg.mdt
    T, GRP, NT, NG, NCHUNK = cfg.T, cfg.GRP, cfg.NT, cfg.NG, cfg.NCHUNK
    DIN, DHID, DOUT, ROWS, Q = cfg.DIN, cfg.DHID, cfg.DOUT, cfg.ROWS, cfg.QUARTER
    region = meta["region"]
    seg_len = meta["seg_len"]
    slot_base = meta["slot_base"]
    total_slots = meta["total_slots"]
    total_blocks = meta["total_blocks"]
    max_gslots = int(meta["group_slots"].max())
    max_gblocks = max_gslots // 128

    # ---- kernel I/O ----
    h0 = [
        nc.dram_tensor(f"h0_{q}", [cfg.CHUNK_ROWS, DIN], mdt, kind="ExternalInput")
        for q in range(NCHUNK)
    ]
    IDX = nc.dram_tensor("IDX", [128, total_slots // 16], I16, kind="ExternalInput")
    DSTL = nc.dram_tensor("DSTL", [128, total_blocks], mdt, kind="ExternalInput")
    WGT = nc.dram_tensor("WGT", [128, total_blocks], mdt, kind="ExternalInput")
    Ws = [
        nc.dram_tensor("W1", [DIN, DHID], mdt, kind="ExternalInput"),
        nc.dram_tensor("W2", [DHID, DHID], mdt, kind="ExternalInput"),
        nc.dram_tensor("W3", [DHID, DOUT], mdt, kind="ExternalInput"),
    ]
    Bs = [
        nc.dram_tensor("B1T", [128, DHID], F32, kind="ExternalInput"),
        nc.dram_tensor("B2T", [128, DHID], F32, kind="ExternalInput"),
        nc.dram_tensor("B3T", [128, DOUT], F32, kind="ExternalInput"),
    ]
    IOTA = nc.dram_tensor("IOTA", [128, T], mdt, kind="ExternalInput")
    z = nc.dram_tensor("z", [ROWS, DOUT], F32, kind="ExternalOutput")

    replica_groups = [list(range(cfg.C))]

    with tile.TileContext(nc, linearize=linearize) as tc:
        import contextlib

        ctx = contextlib.ExitStack()
        with ctx:
            const = ctx.enter_context(tc.tile_pool(name="const", bufs=1))
            auxp = ctx.enter_context(tc.tile_pool(name="aux", bufs=2))
            gpool = ctx.enter_context(tc.tile_pool(name="gbuf", bufs=cfg.gbufs))
            ohpool = ctx.enter_context(tc.tile_pool(name="oh", bufs=4))
            ytpool = ctx.enter_context(tc.tile_pool(name="yt", bufs=3))
            hpool = ctx.enter_context(tc.tile_pool(name="hout", bufs=3))
            psY = ctx.enter_context(tc.tile_pool(name="psY", bufs=3, space="PSUM"))
            psH = ctx.enter_context(tc.tile_pool(name="psH", bufs=2, space="PSUM"))
            dram = ctx.enter_context(tc.tile_pool(name="dram", bufs=1, space="DRAM"))

            # constants
            w_t = []
            b_t = []
            for li, (W, B) in enumerate(zip(Ws, Bs)):
                wt = const.tile(list(W.shape), mdt, name=f"w{li}")
                nc.sync.dma_start(wt[:], W.ap())
                bt = const.tile(list(B.shape), F32, name=f"b{li}")
                nc.sync.dma_start(bt[:], B.ap())
                w_t.append(wt)
                b_t.append(bt)
            iota_t = const.tile([128, T], mdt, name="iota")
            nc.sync.dma_start(iota_t[:], IOTA.ap())

            # inter-layer tables and slices
            h_next = []
            slices = []
            for li in range(2):
                D = DHID
                tabs = [
                    dram.tile(
                        [cfg.CHUNK_ROWS, D], mdt, addr_space="Shared",
                        name=f"h{li + 1}_{q}",
                    )
                    for q in range(NCHUNK)
                ]
                sl = dram.tile([ROWS, D], mdt, name=f"slice{li + 1}")
                h_next.append(tabs)
                slices.append(sl)

            layer_tabs = [
                [t.ap() for t in h0],
                [t[:] for t in h_next[0]],
                [t[:] for t in h_next[1]],
            ]

            for li in layers:
                tabs = layer_tabs[li]
                W_s = w_t[li]
                B_s = b_t[li]
                DO = DHID if li < 2 else DOUT
                relu = li < 2

                for g in range(NG):
                    gs = int(slot_base[g])
                    g_slots = int(meta["group_slots"][g])
                    if g_slots == 0:
                        continue
                    g_blocks = g_slots // 128

                    idx_t = auxp.tile([128, max_gslots // 16], I16, tag="idx")
                    nc.sync.dma_start(
                        idx_t[:, : g_slots // 16],
                        IDX.ap()[:, gs // 16 : (gs + g_slots) // 16],
                    )
                    dstl_t = auxp.tile([128, max_gblocks], mdt, tag="dstl")
                    nc.sync.dma_start(
                        dstl_t[:, :g_blocks],
                        DSTL.ap()[:, gs // 128 : gs // 128 + g_blocks],
                    )
                    wgt_t = auxp.tile([128, max_gblocks], mdt, tag="wgt")
                    nc.sync.dma_start(
                        wgt_t[:, :g_blocks],
                        WGT.ap()[:, gs // 128 : gs // 128 + g_blocks],
                    )

                    gbuf = gpool.tile([128, max_gblocks, DIN], mdt, tag="g")
                    off = 0
                    for q in range(NCHUNK):
                        L = int(seg_len[g, q])
                        if L == 0:
                            continue
                        nc.gpsimd.dma_gather(
                            gbuf[:, off // 128 : (off + L) // 128, :],
                            tabs[q],
                            idx_t[:, off // 16 : (off + L) // 16],
                            num_idxs=L,
                            num_idxs_reg=L,
                            elem_size=DIN,
                            single_packet=False,
                        )
                        off += L

                    # per destination tile in this group
                    for tl in range(GRP):
                        t_glob = g * GRP + tl
                        if t_glob >= NT:
                            continue
                        m_tile = min(T, ROWS - t_glob * T)
                        # blocks of this tile across the chunk segments
                        blocks = []
                        off = 0
                        for q in range(NCHUNK):
                            roff = off + int(region[g, q, :tl].sum())
                            nb = int(region[g, q, tl]) // 128
                            blocks.extend(
                                range(roff // 128, roff // 128 + nb)
                            )
                            off += int(seg_len[g, q])
                        if not blocks:
                            continue

                        psum_y = psY.tile([128, T], F32, tag="y")
                        for k, blk in enumerate(blocks):
                            oh = ohpool.tile([128, T], mdt, tag="oh")
                            nc.vector.tensor_scalar(
                                oh[:],
                                iota_t[:],
                                dstl_t[:, blk : blk + 1],
                                wgt_t[:, blk : blk + 1],
                                op0=mybir.AluOpType.is_equal,
                                op1=mybir.AluOpType.mult,
                            )
                            lhsT = gbuf[:, blk, :]
                            rhs = oh[:]
                            if cfg.use_f32r:
                                lhsT = lhsT.bitcast(F32R)
                                rhs = rhs.bitcast(F32R)
                            nc.tensor.matmul(
                                psum_y[:],
                                lhsT,
                                rhs,
                                start=(k == 0),
                                stop=(k == len(blocks) - 1),
                            )

                        yt = ytpool.tile([128, T], mdt, tag="yt")
                        nc.scalar.copy(yt[:], psum_y[:])

                        for h in range(math.ceil(m_tile / 128)):
                            m = min(128, m_tile - h * 128)
                            rows0 = t_glob * T + h * 128
                            psum_h = psH.tile([128, DO], F32, tag="h")
                            nc.tensor.matmul(
                                psum_h[:m, :],
                                yt[:, h * 128 : h * 128 + m],
                                W_s[:],
                                start=True,
                                stop=True,
                            )
                            if relu:
                                hf = hpool.tile([128, DO], F32, tag="hf")
                                nc.vector.tensor_add(
                                    hf[:m, :], psum_h[:m, :], B_s[:m, :]
                                )
                                ho = hpool.tile([128, DO], mdt, tag="ho")
                                nc.scalar.activation(
                                    ho[:m, :], hf[:m, :],
                                    mybir.ActivationFunctionType.Relu,
                                )
                                nc.sync.dma_start(
                                    slices[li][rows0 : rows0 + m, :], ho[:m, :]
                                )
                            else:
                                zo = hpool.tile([128, DO], F32, tag="zo")
                                nc.vector.tensor_add(
                                    zo[:m, :], psum_h[:m, :], B_s[:m, :]
                                )
                                nc.sync.dma_start(
                                    z.ap()[rows0 : rows0 + m, :], zo[:m, :]
                                )

                if li < 2:
                    if skip_ag:
                        for q in range(NCHUNK):
                            nc.sync.dma_start(
                                h_next[li][q][: cfg.QUARTER, :],
                                slices[li][q * Q : (q + 1) * Q, :],
                            )
                    else:
                        for q in range(NCHUNK):
                            nc.gpsimd.collective_compute(
                                "AllGather",
                                mybir.AluOpType.bypass,
                                replica_groups=replica_groups,
                                ins=[slices[li][q * Q : (q + 1) * Q, :].opt()],
                                outs=[h_next[li][q][:].opt()],
                            )
    nc.compile()
    return nc


def _make_iota(cfg):
    return np.tile(np.arange(cfg.T, dtype=np.float32), (128, 1)).astype(cfg.np_mdt)


def run_gcn(cfg, features, edge_src, edge_dst, edge_weight, W1, b1, W2, b2, W3, b3,
            trace=False, linearize=False):
    per_core_aux, meta = _prep(cfg, edge_src, edge_dst, edge_weight)
    tabs = _chunk_tables(cfg, np.asarray(features, np.float32).astype(cfg.np_mdt))

    nc = bacc.Bacc("TRN2", target_bir_lowering=False, debug=False)
    _build(nc, cfg, meta, linearize=linearize)

    iota = _make_iota(cfg)
    shared = {
        "W1": np.asarray(W1, np.float32).astype(cfg.np_mdt),
        "W2": np.asarray(W2, np.float32).astype(cfg.np_mdt),
        "W3": np.asarray(W3, np.float32).astype(cfg.np_mdt),
        "B1T": np.tile(np.asarray(b1, np.float32), (128, 1)),
        "B2T": np.tile(np.asarray(b2, np.float32), (128, 1)),
        "B3T": np.tile(np.asarray(b3, np.float32), (128, 1)),
        "IOTA": iota,
    }
    for q in range(cfg.NCHUNK):
        shared[f"h0_{q}"] = tabs[q]

    in_maps = [dict(shared, **per_core_aux[c]) for c in range(cfg.C)]
    res = run_bass_kernel_spmd(
        nc, in_maps, list(range(cfg.C)), trace=trace,
    )
    out = np.concatenate([res.results[c]["z"] for c in range(cfg.C)], axis=0)
    return out, res


def kernel(features, edge_src, edge_dst, edge_weight, W1, b1, W2, b2, W3, b3):
    cfg = Cfg(
        n_nodes=100000, d_in=128, d_hid=128, d_out=64,
        n_cores=8, tile_w=256, grp=2, n_chunks=4, dt="f32", use_f32r=True,
    )
    out, _ = run_gcn(
        cfg, features, edge_src, edge_dst, edge_weight, W1, b1, W2, b2, W3, b3,
    )
    return out


# revision 12
# speedup vs baseline: 1.7794x; 1.7794x over previous
"""3-layer GCN (SpMM -> GEMM -> bias -> relu, x3) on 8 Trainium2 NeuronCores.

Strategy (pull-mode graph parallelism):
  - Destination nodes are sharded across the 8 cores (12500 rows each); edges
    are partitioned by destination core.  W/b weights are replicated.
  - Each layer's node-feature table is replicated in every core's DRAM,
    split into NCHUNK row-chunks so dma_gather's int16 indices can address
    any row.  Chunk q holds, for every rank r, rank r's q-th quarter of
    rows, which is exactly what a per-quarter AllGather produces.
  - SpMM is computed per 128/256-wide destination tile: gather the source
    rows for the tile's edges (dma_gather), build a weighted one-hot
    [edges x T] matrix on the Vector engine (iota == dst_local) * w, and
    accumulate  Y^T[feat, dst] += G[edges, feat].T @ onehot[edges, dst]
    on the TensorEngine in PSUM.
  - The dense layer is then  H[dst, dout] = (Y^T).T @ W  with Y^T as the
    stationary operand, bias added on the Vector engine, ReLU on Scalar.
  - Between layers the per-core row slices are AllGathered (one collective
    per chunk, so communication overlaps the tail of the layer).
"""

import math

import numpy as np

import concourse.bacc as bacc
import concourse.mybir as mybir
import concourse.tile as tile
from concourse.bass_utils import run_bass_kernel_spmd

F32 = mybir.dt.float32
F32R = mybir.dt.float32r
BF16 = mybir.dt.bfloat16
I16 = mybir.dt.int16


class Cfg:
    def __init__(
        self,
        n_nodes,
        d_in,
        d_hid,
        d_out,
        n_cores=8,
        tile_w=256,
        grp=2,
        n_chunks=4,
        dt="f32",
        use_f32r=True,
        gbufs=2,
        n_queues=4,
    ):
        self.n_queues = n_queues
        self.N = n_nodes
        self.DIN = d_in
        self.DHID = d_hid
        self.DOUT = d_out
        self.C = n_cores
        assert n_nodes % n_cores == 0
        self.ROWS = n_nodes // n_cores
        assert self.ROWS % n_chunks == 0
        self.NCHUNK = n_chunks
        self.QUARTER = self.ROWS // n_chunks
        self.CHUNK_ROWS = n_nodes // n_chunks
        assert self.CHUNK_ROWS <= 32767, "dma_gather idx is int16"
        self.T = tile_w
        self.GRP = grp
        self.NT = math.ceil(self.ROWS / tile_w)
        self.NG = math.ceil(self.NT / grp)
        self.dt = dt
        self.mdt = F32 if dt == "f32" else BF16
        self.np_mdt = np.float32 if dt == "f32" else None  # set below
        self.use_f32r = use_f32r and dt == "f32"
        self.gbufs = gbufs
        if dt != "f32":
            import ml_dtypes

            self.np_mdt = ml_dtypes.bfloat16


def _prep(cfg, edge_src, edge_dst, edge_weight):
    """Host-side edge preprocessing.  Returns per-core input arrays plus the
    static (core-independent) block structure."""
    src = np.asarray(edge_src, dtype=np.int64)
    dst = np.asarray(edge_dst, dtype=np.int64)
    w = np.asarray(edge_weight, dtype=np.float32)

    C, ROWS, T, GRP, Q = cfg.C, cfg.ROWS, cfg.T, cfg.GRP, cfg.QUARTER
    NT, NG, NCHUNK = cfg.NT, cfg.NG, cfg.NCHUNK

    core = dst // ROWS
    dstl_all = dst - core * ROWS
    tl_all = dstl_all // T
    grp_all = tl_all // GRP
    q_all = (src % ROWS) // Q
    cidx_all = (src // ROWS) * Q + (src % Q)

    # counts[c, g, q, t_local]
    counts = np.zeros((C, NG, NCHUNK, GRP), dtype=np.int64)
    tloc_all = tl_all - grp_all * GRP
    np.add.at(
        counts,
        (core, grp_all, q_all, tloc_all),
        1,
    )
    # static region sizes: equal across cores, multiple of 128 per region
    region = counts.max(axis=0)  # [NG, NCHUNK, GRP]
    region = ((region + 127) // 128) * 128
    # tiles past NT contribute nothing (last group may be partial)
    for g in range(NG):
        for tl in range(GRP):
            if g * GRP + tl >= NT:
                region[g, :, tl] = 0

    seg_len = region.sum(axis=2)  # [NG, NCHUNK] slots per gather call
    group_slots = seg_len.sum(axis=1)  # [NG]
    slot_base = np.concatenate([[0], np.cumsum(group_slots)])
    total_slots = int(slot_base[-1])

    # slot offset of region (g, q, t_local)
    reg_off = np.zeros((NG, NCHUNK, GRP), dtype=np.int64)
    for g in range(NG):
        off = slot_base[g]
        for q in range(NCHUNK):
            for tl in range(GRP):
                reg_off[g, q, tl] = off
                off += region[g, q, tl]

    # per-core slot arrays
    per_core = []
    for c in range(C):
        m = core == c
        e_cidx = cidx_all[m]
        e_q = q_all[m]
        e_g = grp_all[m]
        e_tl = tloc_all[m]
        e_dstl = dstl_all[m] - (e_g * GRP + e_tl) * T  # local within tile
        e_w = w[m]
        order = np.lexsort((e_tl, e_q, e_g))
        e_cidx, e_q, e_g, e_tl, e_dstl, e_w = (
            a[order] for a in (e_cidx, e_q, e_g, e_tl, e_dstl, e_w)
        )
        # position within its (g,q,t) region
        cnt = counts[c, e_g, e_q, e_tl]
        # running index within each sorted group-run
        # since sorted by (g,q,t), positions are 0..cnt-1 per run
        run_starts = np.flatnonzero(
            np.concatenate(
                [[True], (np.diff(e_g) != 0) | (np.diff(e_q) != 0) | (np.diff(e_tl) != 0)]
            )
        )
        pos = np.arange(len(e_g)) - np.repeat(
            run_starts, np.diff(np.concatenate([run_starts, [len(e_g)]]))
        )
        assert (pos < cnt).all()
        slots = reg_off[e_g, e_q, e_tl] + pos

        gidx = np.zeros(total_slots, dtype=np.int16)
        dstl_arr = np.full(total_slots, -1.0, dtype=np.float32)
        w_arr = np.zeros(total_slots, dtype=np.float32)
        gidx[slots] = e_cidx.astype(np.int16)
        dstl_arr[slots] = e_dstl.astype(np.float32)
        w_arr[slots] = e_w
        per_core.append((gidx, dstl_arr, w_arr))

    # pack into DMA-ready layouts
    total_blocks = total_slots // 128
    inputs = []
    for c in range(C):
        gidx, dstl_arr, w_arr = per_core[c]
        # idx: per (g,q) segment, wrapped [16, L/16], replicated to 128 parts
        idx_plane = np.zeros((128, total_slots // 16), dtype=np.int16)
        for g in range(NG):
            off = int(slot_base[g])
            for q in range(NCHUNK):
                L = int(seg_len[g, q])
                if L == 0:
                    continue
                seg = gidx[off : off + L]
                wrapped = seg.reshape(L // 16, 16).T  # [16, L/16]
                idx_plane[:, off // 16 : (off + L) // 16] = np.tile(wrapped, (8, 1))
                off += L
        dstl_plane = dstl_arr.reshape(total_blocks, 128).T.astype(cfg.np_mdt)
        w_plane = w_arr.reshape(total_blocks, 128).T.astype(cfg.np_mdt)
        inputs.append(
            {"IDX": idx_plane, "DSTL": np.ascontiguousarray(dstl_plane),
             "WGT": np.ascontiguousarray(w_plane)}
        )

    meta = {
        "region": region,
        "seg_len": seg_len,
        "group_slots": group_slots,
        "slot_base": slot_base,
        "reg_off": reg_off,
        "total_slots": total_slots,
        "total_blocks": total_blocks,
    }
    return inputs, meta


def _chunk_tables(cfg, h):
    """Rearrange full [N, D] host array into NCHUNK chunk tables
    [CHUNK_ROWS, D]: chunk q row (r*QUARTER + j) = node r*ROWS + q*QUARTER + j."""
    C, ROWS, Q = cfg.C, cfg.ROWS, cfg.QUARTER
    tabs = []
    for q in range(cfg.NCHUNK):
        rows = np.concatenate(
            [h[r * ROWS + q * Q : r * ROWS + (q + 1) * Q] for r in range(C)], axis=0
        )
        tabs.append(np.ascontiguousarray(rows))
    return tabs


def _build(nc, cfg, meta, linearize=False, layers=(0, 1, 2), skip_ag=False):
    """Emit the 3-layer GCN program for one core (SPMD: all cores identical)."""
    mdt = cfg.mdt
    T, GRP, NT, NG, NCHUNK = cfg.T, cfg.GRP, cfg.NT, cfg.NG, cfg.NCHUNK
    DIN, DHID, DOUT, ROWS, Q = cfg.DIN, cfg.DHID, cfg.DOUT, cfg.ROWS, cfg.QUARTER
    region = meta["region"]
    seg_len = meta["seg_len"]
    slot_base = meta["slot_base"]
    total_slots = meta["total_slots"]
    total_blocks = meta["total_blocks"]
    max_gslots = int(meta["group_slots"].max())
    max_gblocks = max_gslots // 128

    # ---- kernel I/O ----
    h0 = [
        nc.dram_tensor(f"h0_{q}", [cfg.CHUNK_ROWS, DIN], mdt, kind="ExternalInput")
        for q in range(NCHUNK)
    ]
    IDX = nc.dram_tensor("IDX", [128, total_slots // 16], I16, kind="ExternalInput")
    DSTL = nc.dram_tensor("DSTL", [128, total_blocks], mdt, kind="ExternalInput")
    WGT = nc.dram_tensor("WGT", [128, total_blocks], mdt, kind="ExternalInput")
    Ws = [
        nc.dram_tensor("W1", [DIN, DHID], mdt, kind="ExternalInput"),
        nc.dram_tensor("W2", [DHID, DHID], mdt, kind="ExternalInput"),
        nc.dram_tensor("W3", [DHID, DOUT], mdt, kind="ExternalInput"),
    ]
    Bs = [
        nc.dram_tensor("B1T", [128, DHID], F32, kind="ExternalInput"),
        nc.dram_tensor("B2T", [128, DHID], F32, kind="ExternalInput"),
        nc.dram_tensor("B3T", [128, DOUT], F32, kind="ExternalInput"),
    ]
    IOTA = nc.dram_tensor("IOTA", [128, T], mdt, kind="ExternalInput")
    z = nc.dram_tensor("z", [ROWS, DOUT], F32, kind="ExternalOutput")

    replica_groups = [list(range(cfg.C))]

    with tile.TileContext(nc, linearize=linearize) as tc:
        import contextlib

        ctx = contextlib.ExitStack()
        with ctx:
            const = ctx.enter_context(tc.tile_pool(name="const", bufs=1))
            auxp = ctx.enter_context(tc.tile_pool(name="aux", bufs=2))
            gpool = ctx.enter_context(tc.tile_pool(name="gbuf", bufs=cfg.gbufs))
            ohpool = ctx.enter_context(tc.tile_pool(name="oh", bufs=4))
            ytpool = ctx.enter_context(tc.tile_pool(name="yt", bufs=3))
            hpool = ctx.enter_context(tc.tile_pool(name="hout", bufs=3))
            psY = ctx.enter_context(tc.tile_pool(name="psY", bufs=3, space="PSUM"))
            psH = ctx.enter_context(tc.tile_pool(name="psH", bufs=2, space="PSUM"))
            dram = ctx.enter_context(tc.tile_pool(name="dram", bufs=1, space="DRAM"))

            # constants
            w_t = []
            b_t = []
            for li, (W, B) in enumerate(zip(Ws, Bs)):
                wt = const.tile(list(W.shape), mdt, name=f"w{li}")
                nc.sync.dma_start(wt[:], W.ap())
                bt = const.tile(list(B.shape), F32, name=f"b{li}")
                nc.sync.dma_start(bt[:], B.ap())
                w_t.append(wt)
                b_t.append(bt)
            iota_t = const.tile([128, T], mdt, name="iota")
            nc.sync.dma_start(iota_t[:], IOTA.ap())

            # inter-layer tables and slices
            h_next = []
            slices = []
            for li in range(2):
                D = DHID
                tabs = [
                    dram.tile(
                        [cfg.CHUNK_ROWS, D], mdt, addr_space="Shared",
                        name=f"h{li + 1}_{q}",
                    )
                    for q in range(NCHUNK)
                ]
                sl = dram.tile([ROWS, D], mdt, name=f"slice{li + 1}")
                h_next.append(tabs)
                slices.append(sl)

            layer_tabs = [
                [t.ap() for t in h0],
                [t[:] for t in h_next[0]],
                [t[:] for t in h_next[1]],
            ]

            for li in layers:
                tabs = layer_tabs[li]
                W_s = w_t[li]
                B_s = b_t[li]
                DO = DHID if li < 2 else DOUT
                relu = li < 2

                for g in range(NG):
                    gs = int(slot_base[g])
                    g_slots = int(meta["group_slots"][g])
                    if g_slots == 0:
                        continue
                    g_blocks = g_slots // 128

                    idx_t = auxp.tile([128, max_gslots // 16], I16, tag="idx")
                    nc.sync.dma_start(
                        idx_t[:, : g_slots // 16],
                        IDX.ap()[:, gs // 16 : (gs + g_slots) // 16],
                    )
                    dstl_t = auxp.tile([128, max_gblocks], mdt, tag="dstl")
                    nc.sync.dma_start(
                        dstl_t[:, :g_blocks],
                        DSTL.ap()[:, gs // 128 : gs // 128 + g_blocks],
                    )
                    wgt_t = auxp.tile([128, max_gblocks], mdt, tag="wgt")
                    nc.sync.dma_start(
                        wgt_t[:, :g_blocks],
                        WGT.ap()[:, gs // 128 : gs // 128 + g_blocks],
                    )

                    gbuf = gpool.tile([128, max_gblocks, DIN], mdt, tag="g")
                    off = 0
                    for q in range(NCHUNK):
                        L = int(seg_len[g, q])
                        if L == 0:
                            continue
                        nc.gpsimd.dma_gather(
                            gbuf[:, off // 128 : (off + L) // 128, :],
                            tabs[q],
                            idx_t[:, off // 16 : (off + L) // 16],
                            num_idxs=L,
                            num_idxs_reg=L,
                            elem_size=DIN,
                            single_packet=False,
                            queue_num=q % cfg.n_queues,
                        )
                        off += L

                    # per destination tile in this group
                    for tl in range(GRP):
                        t_glob = g * GRP + tl
                        if t_glob >= NT:
                            continue
                        m_tile = min(T, ROWS - t_glob * T)
                        # blocks of this tile across the chunk segments
                        blocks = []
                        off = 0
                        for q in range(NCHUNK):
                            roff = off + int(region[g, q, :tl].sum())
                            nb = int(region[g, q, tl]) // 128
                            blocks.extend(
                                range(roff // 128, roff // 128 + nb)
                            )
                            off += int(seg_len[g, q])
                        if not blocks:
                            continue

                        psum_y = psY.tile([128, T], F32, tag="y")
                        oh_dt = F32R if cfg.use_f32r else mdt
                        for k, blk in enumerate(blocks):
                            oh = ohpool.tile([128, T], oh_dt, tag="oh")
                            nc.vector.tensor_scalar(
                                oh[:],
                                iota_t[:],
                                dstl_t[:, blk : blk + 1],
                                wgt_t[:, blk : blk + 1],
                                op0=mybir.AluOpType.is_equal,
                                op1=mybir.AluOpType.mult,
                            )
                            nc.tensor.matmul(
                                psum_y[:],
                                gbuf[:, blk, :],
                                oh[:],
                                start=(k == 0),
                                stop=(k == len(blocks) - 1),
                            )

                        yt = ytpool.tile([128, T], mdt, tag="yt")
                        nc.scalar.copy(yt[:], psum_y[:])

                        for h in range(math.ceil(m_tile / 128)):
                            m = min(128, m_tile - h * 128)
                            rows0 = t_glob * T + h * 128
                            psum_h = psH.tile([128, DO], F32, tag="h")
                            nc.tensor.matmul(
                                psum_h[:m, :],
                                yt[:, h * 128 : h * 128 + m],
                                W_s[:],
                                start=True,
                                stop=True,
                            )
                            if relu:
                                hf = hpool.tile([128, DO], F32, tag="hf")
                                nc.vector.tensor_add(
                                    hf[:m, :], psum_h[:m, :], B_s[:m, :]
                                )
                                ho = hpool.tile([128, DO], mdt, tag="ho")
                                nc.scalar.activation(
                                    ho[:m, :], hf[:m, :],
                                    mybir.ActivationFunctionType.Relu,
                                )
                                nc.sync.dma_start(
                                    slices[li][rows0 : rows0 + m, :], ho[:m, :]
                                )
                            else:
                                zo = hpool.tile([128, DO], F32, tag="zo")
                                nc.vector.tensor_add(
                                    zo[:m, :], psum_h[:m, :], B_s[:m, :]
                                )
                                nc.sync.dma_start(
                                    z.ap()[rows0 : rows0 + m, :], zo[:m, :]
                                )

                if li < 2:
                    if skip_ag:
                        for q in range(NCHUNK):
                            nc.sync.dma_start(
                                h_next[li][q][: cfg.QUARTER, :],
                                slices[li][q * Q : (q + 1) * Q, :],
                            )
                    else:
                        for q in range(NCHUNK):
                            nc.gpsimd.collective_compute(
                                "AllGather",
                                mybir.AluOpType.bypass,
                                replica_groups=replica_groups,
                                ins=[slices[li][q * Q : (q + 1) * Q, :].opt()],
                                outs=[h_next[li][q][:].opt()],
                            )
    nc.compile()
    return nc


def _make_iota(cfg):
    return np.tile(np.arange(cfg.T, dtype=np.float32), (128, 1)).astype(cfg.np_mdt)


def run_gcn(cfg, features, edge_src, edge_dst, edge_weight, W1, b1, W2, b2, W3, b3,
            trace=False, linearize=False):
    per_core_aux, meta = _prep(cfg, edge_src, edge_dst, edge_weight)
    tabs = _chunk_tables(cfg, np.asarray(features, np.float32).astype(cfg.np_mdt))

    nc = bacc.Bacc(
        "TRN2", target_bir_lowering=False, debug=False,
        num_swdge_queues=cfg.n_queues,
    )
    _build(nc, cfg, meta, linearize=linearize)

    iota = _make_iota(cfg)
    shared = {
        "W1": np.asarray(W1, np.float32).astype(cfg.np_mdt),
        "W2": np.asarray(W2, np.float32).astype(cfg.np_mdt),
        "W3": np.asarray(W3, np.float32).astype(cfg.np_mdt),
        "B1T": np.tile(np.asarray(b1, np.float32), (128, 1)),
        "B2T": np.tile(np.asarray(b2, np.float32), (128, 1)),
        "B3T": np.tile(np.asarray(b3, np.float32), (128, 1)),
        "IOTA": iota,
    }
    for q in range(cfg.NCHUNK):
        shared[f"h0_{q}"] = tabs[q]

    in_maps = [dict(shared, **per_core_aux[c]) for c in range(cfg.C)]
    res = run_bass_kernel_spmd(
        nc, in_maps, list(range(cfg.C)), trace=trace,
    )
    out = np.concatenate([res.results[c]["z"] for c in range(cfg.C)], axis=0)
    return out, res


def kernel(features, edge_src, edge_dst, edge_weight, W1, b1, W2, b2, W3, b3):
    cfg = Cfg(
        n_nodes=100000, d_in=128, d_hid=128, d_out=64,
        n_cores=8, tile_w=256, grp=2, n_chunks=4, dt="f32", use_f32r=True,
    )
    out, _ = run_gcn(
        cfg, features, edge_src, edge_dst, edge_weight, W1, b1, W2, b2, W3, b3,
    )
    return out
